# revision 52
# baseline (speedup 1.0000x reference)
"""Trainium2 Bass kernel for nn_DE_TGraph (diachronic temporal-graph GNN layer).

Strategy (8 NeuronCores, SPMD, relation-sharded):
  - 460 relations partitioned across 8 cores by size-rank snake order.
    Per-relation BatchNorm is core-local (no collectives for BN stats).
  - Host packs per-neighbor feature rows (ent96 | freq96 | phi96 | amp96)
    in slot-column order per core, so the device loads them with plain
    contiguous DMA (no gpsimd table gather on the critical path).  Slots
    are greedily packed into 512-column blocks (PSUM-bank sized).
  - Time embedding on DVE/ACT (sin on ScalarE) in row layout per
    8-chunk group; x chunks transposed on PE into xT.
  - Pass 1: per-slot GEMM z^T = W_r^T xT into a shared per-block PSUM
    bank + bn_stats per slot; one bf16 copy per block saves z to SBUF
    (no second GEMM pass).
  - BN affine: per batch of blocks, coefficients a,c are computed on
    DVE, transposed on PE, and expanded to per-column vectors via small
    K=|batch slots| matmuls against a host 0/1 expansion matrix;
    zn = relu(a_exp*z + c_exp) via two DVE block ops + one ACT relu
    per 512 block.
  - Per 4-chunk block: PE transposes zn -> z rows (copies rotate over
    ACT/gpsimd/DVE), z_d flushed per block; edge-tile dma_gathers fire
    as soon as their z_d watermark is met (gpsimd is otherwise idle),
    and D^T[d,b] += zg^T @ P_t accumulates in PSUM.  D^T is this core's
    partial of emb[head]-emb[tail]; rel_embs[rels]^T/8 folded in.
  - Warm-up AllReduce at kernel start absorbs CC firmware spin-up; one
    real AllReduce of D^T [128, B] bf16.
  - Scoring: score = -sqrt(sum_d v^2) via square + ones-vector matmul.
"""
import numpy as np

import concourse.bacc as bacc
import concourse.mybir as mybir
import concourse.tile as tile
from concourse import library_config
from concourse.bass_utils import run_bass_kernel_spmd

F32 = mybir.dt.float32
BF16 = mybir.dt.bfloat16
DT16 = BF16
import ml_dtypes
NPDT16 = ml_dtypes.bfloat16
I16 = mybir.dt.int16
AF = mybir.ActivationFunctionType
OP = mybir.AluOpType

NUM_ENT = 10000
NUM_REL = 230
R2 = 2 * NUM_REL
S_DIM = 96
T_DIM = 32
DIM = 128
N = 32768
Q = 4096
E = 32768
B = 1024
NC = 8
BN_EPS = 1e-5
BLK = 512          # PSUM-bank block (columns)
BCH = BLK // 128   # chunks per block (4)
NBATCH = 3         # coef batches

# Probed XOR-peer map: BETA[a][d] = logical rank of core a's remote_dma
# peer at relative Δtpb=d (hardware XORs *physical* tpb; the driver's
# logical->physical NC map is identity on die 0 and ^2 on die 1).
# BETA[a][j] is also the global triple block held at core a's local
# block j in the butterfly exchange.
BETA = (
    (0, 1, 2, 3, 6, 7, 4, 5),
    (1, 0, 3, 2, 7, 6, 5, 4),
    (2, 3, 0, 1, 4, 5, 6, 7),
    (3, 2, 1, 0, 5, 4, 7, 6),
    (4, 5, 6, 7, 2, 3, 0, 1),
    (5, 4, 7, 6, 3, 2, 1, 0),
    (6, 7, 4, 5, 0, 1, 2, 3),
    (7, 6, 5, 4, 1, 0, 3, 2),
)


def _wrap16(idx, n):
    """[n] int -> [128, n//16] int16 (index i at [i%16, i//16], replicated)."""
    assert n % 16 == 0 and len(idx) == n
    w = np.asarray(idx).reshape(n // 16, 16).T.astype(np.int16)
    return np.ascontiguousarray(np.tile(w, (8, 1)))


def _tile_rows(x, p=128):
    """[n, d] -> [128, n//p, d] tile layout (row c*128+p -> [p, c, :])."""
    n, d = x.shape
    assert n % p == 0
    return np.ascontiguousarray(x.reshape(n // p, p, d).transpose(1, 0, 2))


class _Plan:
    pass


def _make_plan(inp):
    p = _Plan()
    rel = np.asarray(inp["rel_id"]).astype(np.int64).reshape(-1)
    nidx = np.asarray(inp["neighbor_idx"]).astype(np.int64).reshape(-1)
    years = np.asarray(inp["years"], np.float32).reshape(-1)
    months = np.asarray(inp["months"], np.float32).reshape(-1)
    days = np.asarray(inp["days"], np.float32).reshape(-1)
    psrc = np.asarray(inp["pool_src"]).astype(np.int64).reshape(-1)
    pdst = np.asarray(inp["pool_dst"]).astype(np.int64).reshape(-1)
    head = np.asarray(inp["head_pos"]).astype(np.int64).reshape(-1)
    tail = np.asarray(inp["tail_pos"]).astype(np.int64).reshape(-1)
    rels = np.asarray(inp["rels"]).astype(np.int64).reshape(-1)
    W = np.asarray(inp["W"], np.float32)
    b = np.asarray(inp["b"], np.float32)
    gamma = np.asarray(inp["gamma"], np.float32)
    beta = np.asarray(inp["beta"], np.float32)

    # ---- relation partition: size-rank snake across cores ----
    cnts = np.bincount(rel, minlength=R2)
    order = np.argsort(-cnts, kind="stable")
    SLOTS = (R2 + NC - 1) // NC
    core_rels = [[None] * SLOTS for _ in range(NC)]
    caps = np.zeros(SLOTS, np.int64)
    for s in range(SLOTS):
        grp = order[s * NC : (s + 1) * NC]
        caps[s] = cnts[grp[0]]  # exact group max; only block tails padded
        perm = range(NC) if s % 2 == 0 else range(NC - 1, -1, -1)
        for r, c in zip(grp, perm):
            core_rels[c][s] = int(r)
    assert caps.max() <= BLK, "relation bucket exceeds 512 (unsupported)"

    # greedy-pack slots into 512-col blocks; each block padded to 512
    col_start = np.zeros(SLOTS + 1, np.int64)
    blk_slots = []  # (s0, s1) slot range per block
    cur = 0
    s0 = 0
    for s in range(SLOTS):
        if cur % BLK + caps[s] > BLK:
            blk_slots.append((s0, s))
            s0 = s
            cur = (cur // BLK + 1) * BLK
        col_start[s] = cur
        cur += int(caps[s])
    blk_slots.append((s0, SLOTS))
    col_start[SLOTS] = cur
    NCOLS = int(cur)
    NBLK = len(blk_slots)
    N_cap = NBLK * BLK
    C_n = N_cap // 128

    # coef batches over block ranges (host + device must agree)
    bper = (NBLK + NBATCH - 1) // NBATCH
    batches = tuple((k * bper, min((k + 1) * bper, NBLK))
                    for k in range(NBATCH) if k * bper < NBLK)

    # positions per relation
    order_by_rel = np.argsort(rel, kind="stable")
    rel_sorted = rel[order_by_rel]
    starts = np.searchsorted(rel_sorted, np.arange(R2))
    ends = np.searchsorted(rel_sorted, np.arange(R2), side="right")

    pos_core = np.empty(N, np.int64)
    pos_col = np.empty(N, np.int64)

    # packed per-entity table (bf16): ent96 | yf mf df | yp mp dp | ya ma da
    tbl = np.zeros((NUM_ENT, 384), np.float32)
    tbl[:, 0:96] = np.asarray(inp["ent_embs"], np.float32)
    for k, nm in enumerate(["y_freq", "m_freq", "d_freq",
                            "y_phi", "m_phi", "d_phi",
                            "y_amp", "m_amp", "d_amp"]):
        tbl[:, 96 + 32 * k : 128 + 32 * k] = np.asarray(inp[nm], np.float32)
    tbl16 = tbl.astype(NPDT16)

    xrow_cores, tcat_cores = [], []
    for c in range(NC):
        gidx = np.zeros(N_cap, np.int64)
        valid = np.zeros(N_cap, bool)
        tc3 = np.zeros((N_cap, 3), np.float32)
        for s in range(SLOTS):
            r = core_rels[c][s]
            if r is None:
                continue
            pp = order_by_rel[starts[r] : ends[r]]
            pp = pp[np.argsort(nidx[pp], kind="stable")]
            j0 = int(col_start[s])
            gidx[j0 : j0 + len(pp)] = nidx[pp]
            valid[j0 : j0 + len(pp)] = True
            tc3[j0 : j0 + len(pp), 0] = years[pp]
            tc3[j0 : j0 + len(pp), 1] = months[pp]
            tc3[j0 : j0 + len(pp), 2] = days[pp]
            pos_core[pp] = c
            pos_col[pp] = j0 + np.arange(len(pp))
        xr = np.zeros((N_cap, 384), NPDT16)
        xr[valid] = tbl16[gidx[valid]]
        xrow_cores.append(_tile_rows(xr))
        tcat_cores.append(_tile_rows(tc3).astype(NPDT16))

    # ---- batch-local 0/1 expansion matrix [64, N_cap] ----
    Emat = np.zeros((64, N_cap), np.float32)
    for (b0, b1) in batches:
        S0 = blk_slots[b0][0]
        S1 = blk_slots[b1 - 1][1]
        assert S1 - S0 <= 64
        for s in range(S0, S1):
            Emat[s - S0, int(col_start[s]) : int(col_start[s]) + int(caps[s])] = 1.0
    p.Emat = np.ascontiguousarray(Emat.astype(NPDT16))

    # ---- per-core weights + BN aux ----
    wloc_cores, aux_cores = [], []
    for c in range(NC):
        wl = np.zeros((SLOTS, DIM, DIM), np.float32)
        invcnt = np.ones(SLOTS, np.float32)
        onem = np.ones(SLOTS, np.float32)
        gT_u = np.zeros((DIM, SLOTS), np.float32)
        ubb = np.zeros((DIM, SLOTS), np.float32)
        for s in range(SLOTS):
            r = core_rels[c][s]
            if r is None:
                continue
            wl[s] = W[r]
            cnt = cnts[r]
            invcnt[s] = 1.0 / max(cnt, 1)
            u = 1.0 if cnt > 1 else 0.0
            onem[s] = 1.0 - u
            gT_u[:, s] = gamma[r] * u
            ubb[:, s] = beta[r] * u + b[r] * (1.0 - u)
        aux = np.zeros((128, 4 * SLOTS), np.float32)
        aux[:, 0:SLOTS] = invcnt[None, :]
        aux[:, SLOTS : 2 * SLOTS] = onem[None, :]
        aux[:, 2 * SLOTS : 3 * SLOTS] = gT_u
        aux[:, 3 * SLOTS : 4 * SLOTS] = ubb
        wloc_cores.append(np.ascontiguousarray(
            wl.transpose(1, 0, 2).astype(NPDT16)))
        aux_cores.append(np.ascontiguousarray(aux))

    # ---- pooling edges: keep only dsts referenced by head/tail ----
    pcnt = np.bincount(pdst, minlength=Q).astype(np.float32)
    used = np.zeros(Q, bool)
    used[head] = True
    used[tail] = True
    keep = used[pdst]
    e_core = pos_core[psrc]
    ecols, edsts = [], []
    for c in range(NC):
        m = keep & (e_core == c)
        es, ed = psrc[m], pdst[m]
        o = np.argsort(pos_col[es], kind="stable")  # z_d locality + watermark
        ecols.append(pos_col[es[o]])
        edsts.append(ed[o])

    # dedup edge srcs: one gathered z row per distinct src, P rows summed
    uniq_cores = [np.unique(x) for x in ecols]
    T_E = max(1, max((len(u) + 127) // 128 for u in uniq_cores))
    NE = T_E * 128

    egidx_cores, p_cores = [], []
    tile_ready = np.zeros(T_E, np.int64)  # z_d chunks needed per edge tile
    for c in range(NC):
        ec, ed = ecols[c], edsts[c]
        uniq = uniq_cores[c]
        rows = np.searchsorted(uniq, ec)
        inv = 1.0 / np.maximum(pcnt[ed], 1.0)
        contrib = ((ed[:, None] == head[None, :]).astype(np.float32)
                   - (ed[:, None] == tail[None, :]).astype(np.float32))
        contrib *= inv[:, None]
        P = np.zeros((NE, B), np.float32)
        np.add.at(P, rows, contrib)
        eg = np.zeros(NE, np.int64)
        eg[: len(uniq)] = uniq
        egidx_cores.append(_wrap16(eg, NE))
        p_cores.append(np.ascontiguousarray(
            P.reshape(T_E, 128, B).transpose(1, 0, 2).astype(NPDT16)))
        hi = eg.reshape(T_E, 128).max(axis=1)  # cols sorted -> per-tile max
        tile_ready = np.maximum(tile_ready, hi // 128 + 1)
    p.tile_ready = tuple(int(x) for x in tile_ready)

    # ---- scoring: rel_embs[rels]^T / NC, folded pre-reduce ----
    # Butterfly exchange uses XOR-local triple blocks: on core c, local
    # 128-col block j holds global block c ^ j.  Permute P columns and
    # relgT per core accordingly; core 0's local order is then global.
    relgT = np.asarray(inp["rel_embs"], np.float32)[rels].T / NC  # [128, B]
    p.relgT = np.ascontiguousarray(relgT.astype(NPDT16))
    p.ident = np.ascontiguousarray(np.eye(128, dtype=NPDT16))

    p.SLOTS, p.caps, p.col_start = SLOTS, caps, col_start
    p.NCOLS, p.N_cap, p.C_n, p.T_E = NCOLS, N_cap, C_n, T_E
    p.NBLK, p.blk_slots, p.batches = NBLK, tuple(blk_slots), batches
    p.xrow, p.tcat = xrow_cores, tcat_cores
    p.wloc, p.aux = wloc_cores, aux_cores
    p.egidx, p.P = egidx_cores, p_cores
    return p


def _build(SLOTS, caps, col_start, N_cap, C_n, T_E, tile_ready, blk_slots,
           batches):
    NBLK = len(blk_slots)
    nc = bacc.Bacc(None, target_bir_lowering=False, debug=False,
                   num_devices=NC, num_swdge_queues=2)
    xrow = nc.dram_tensor("xrow", [128, C_n, 384], DT16, kind="ExternalInput")
    tcat = nc.dram_tensor("tcat", [128, C_n, 3], DT16, kind="ExternalInput")
    wloc = nc.dram_tensor("wloc", [128, SLOTS, DIM], DT16,
                          kind="ExternalInput")
    aux = nc.dram_tensor("aux", [128, 4 * SLOTS], F32, kind="ExternalInput")
    emat = nc.dram_tensor("emat", [64, N_cap], DT16, kind="ExternalInput")
    egidx = nc.dram_tensor("egidx", [128, T_E * 128 // 16], I16,
                           kind="ExternalInput")
    p_d = nc.dram_tensor("p_d", [128, T_E, B], DT16, kind="ExternalInput")
    relgT = nc.dram_tensor("relgT", [128, B], DT16, kind="ExternalInput")
    identd = nc.dram_tensor("identd", [128, 128], DT16, kind="ExternalInput")
    outd = nc.dram_tensor("out", [1, B], F32, kind="ExternalOutput")

    z_d = nc.dram_tensor("z_d", [N_cap, DIM], DT16)
    ard_in = nc.dram_tensor("ard_in", [128, B], DT16)
    ard_out = nc.dram_tensor("ard_out", [128, B], DT16, addr_space="Shared")
    war_in = nc.dram_tensor("war_in", [128, 16], DT16)
    war_out = nc.dram_tensor("war_out", [128, 16], DT16, addr_space="Shared")

    GG = 8  # chunks per x-load / time-embedding group
    n_gg = (C_n + GG - 1) // GG


    with tile.TileContext(nc) as tc:
        with (
            tc.tile_pool(name="pers", bufs=1) as sm,
            tc.tile_pool(name="ps", bufs=2, space="PSUM") as ps,
            tc.tile_pool(name="ptp", bufs=2, space="PSUM") as ptp,
            tc.tile_pool(name="pep", bufs=2, space="PSUM") as pep,
            tc.tile_pool(name="dtp", bufs=1, space="PSUM") as dtp,
            tc.tile_pool(name="ph2", bufs=1) as bg2,
        ):
            nc.gpsimd.load_library(library_config.mlp)

            def load(pool, name, dram, shape, dtype=F32):
                t = pool.tile(shape, dtype, tag=name, name=name)
                nc.sync.dma_start(out=t[:], in_=dram[:])
                return t

            # warm-up collective: pays the CC firmware spin-up cost and
            # re-synchronizes the cores early, off the critical path
            wu = sm.tile([128, 16], DT16, tag="wu")
            nc.vector.memset(wu[:], 0.0)
            nc.sync.dma_start(out=war_in[:], in_=wu[:])
            nc.gpsimd.collective_compute(
                "AllReduce", OP.add,
                replica_groups=[list(range(NC))],
                ins=[war_in.ap().opt()], outs=[war_out.ap().opt()])
            # x rows arrive per 8-chunk group (first-needed-first DMA order)
            xr_t = sm.tile([128, C_n, 384], DT16, tag="xr")
            for g in range(n_gg):
                c0, c1 = g * GG, min((g + 1) * GG, C_n)
                nc.sync.dma_start(out=xr_t[:, c0:c1, :],
                                  in_=xrow[:, c0:c1, :])
            tcat_t = load(sm, "tcat", tcat, [128, C_n, 3], DT16)
            ident16 = load(sm, "identd", identd, [128, 128], DT16)
            w_sb = load(sm, "w_sb", wloc, [128, SLOTS, DIM], DT16)
            aux_t = load(sm, "aux", aux, [128, 4 * SLOTS])
            emat_t = load(sm, "emat", emat, [64, N_cap], DT16)
            egidx_t = load(sm, "egidx", egidx, [128, T_E * 128 // 16], I16)
            p_sb = load(sm, "p_sb", p_d, [128, T_E, B], DT16)
            relgT_t = load(sm, "relgT", relgT, [128, B], DT16)

            xT = sm.tile([128, N_cap], DT16)
            z_sb = sm.tile([128, N_cap], DT16)
            znT = sm.tile([128, N_cap], DT16)
            stats6 = sm.tile([128, SLOTS, 6], F32)
            a_t = sm.tile([128, SLOTS], F32, tag="a_t")
            c_t = sm.tile([128, SLOTS], F32, tag="c_t")
            a16 = sm.tile([128, 128], DT16, tag="a16")
            c16 = sm.tile([128, 128], DT16, tag="c16")
            nc.vector.memset(a16[:], 0.0)
            nc.vector.memset(c16[:], 0.0)
            sc = [sm.tile([128, SLOTS], F32, tag=f"sc{i}", name=f"sc{i}")
                  for i in range(6)]
            V = nc.vector

            dt0 = dtp.tile([128, 512], F32, tag="dt0", space="PSUM",
                           name="dt0")
            dt1 = dtp.tile([128, 512], F32, tag="dt1", space="PSUM",
                           name="dt1")

            # ---- helpers ------------------------------------------------
            cp_eng = [0]

            def next_copy(out, in_):
                # rotate PSUM->SBUF copies over ACT / DVE (gpsimd can't
                # read PSUM)
                k = cp_eng[0] % 2
                cp_eng[0] += 1
                if k == 0:
                    nc.scalar.copy(out=out, in_=in_)
                else:
                    nc.vector.tensor_copy(out=out, in_=in_)

            def temb_group(g):
                # time embedding for chunks [c0,c1): x[:, :, 96:128] final
                c0, c1 = g * GG, min((g + 1) * GG, C_n)
                w = c1 - c0
                gb = xr_t[:, c0:c1, :]
                xs = bg2.tile([128, GG, 96], DT16, tag="xs", name="xs",
                              bufs=2)
                f4 = gb[:, :, 96:192].rearrange("p c (k e) -> p c k e", k=3)
                x4 = xs[:, :w, :].rearrange("p c (k e) -> p c k e", k=3)
                t4 = tcat_t[:, c0:c1, :].unsqueeze(3).to_broadcast(
                    [128, w, 3, T_DIM])
                V.tensor_tensor(out=x4, in0=f4, in1=t4, op=OP.mult)
                V.tensor_tensor(out=xs[:, :w, :], in0=xs[:, :w, :],
                                in1=gb[:, :, 192:288], op=OP.add)
                nc.scalar.activation(out=xs[:, :w, :], in_=xs[:, :w, :],
                                     func=AF.Sin)
                V.tensor_tensor(out=xs[:, :w, :], in0=xs[:, :w, :],
                                in1=gb[:, :, 288:384], op=OP.mult)
                V.tensor_tensor(out=gb[:, :, 96:128], in0=xs[:, :w, 0:32],
                                in1=xs[:, :w, 32:64], op=OP.add)
                V.tensor_tensor(out=gb[:, :, 96:128], in0=gb[:, :, 96:128],
                                in1=xs[:, :w, 64:96], op=OP.add)
                for c in range(c0, c1):
                    pt = ptp.tile([128, 128], DT16, tag="pt", space="PSUM",
                                  name="pt")
                    nc.tensor.transpose(out=pt[:], in_=gb[:, c - c0, 0:128],
                                        identity=ident16[:])
                    next_copy(xT[:, c * 128:(c + 1) * 128], pt[:])

            def pass1_block(b):
                # per-slot GEMM into one shared PSUM bank + stats + z save
                s0, s1 = blk_slots[b]
                base = b * BLK
                zp = ps.tile([128, BLK], F32, tag="zp", space="PSUM",
                             name="zp")
                for s in range(s0, s1):
                    a = int(col_start[s])
                    bb = a + int(caps[s])
                    if s == s1 - 1:
                        bb = base + BLK  # cover block pad (xT cols are zero)
                    nc.tensor.matmul(zp[:, a - base:bb - base],
                                     lhsT=w_sb[:, s, :], rhs=xT[:, a:bb],
                                     start=True, stop=True)
                for s in range(s0, s1):
                    a = int(col_start[s])
                    bb = a + int(caps[s])
                    nc.vector.bn_stats(stats6[:, s, :],
                                       zp[:, a - base:bb - base])
                V.tensor_copy(out=z_sb[:, base:base + BLK], in_=zp[:])

            def coefs(s0, s1):
                # BN coefs a,c for slots [s0,s1) + batch-local bf16 pack
                sl = slice(s0, s1)
                ce, me, ve = (stats6[:, sl, k] for k in (0, 1, 2))
                co, mo, vo = (stats6[:, sl, k] for k in (3, 4, 5))
                invcnt = aux_t[:, s0:s1]
                onem = aux_t[:, SLOTS + s0:SLOTS + s1]
                gT_u = aux_t[:, 2 * SLOTS + s0:2 * SLOTS + s1]
                ubb = aux_t[:, 3 * SLOTS + s0:3 * SLOTS + s1]
                te, to_, s1_, s2, mean, var = (t[:, sl] for t in sc)
                V.tensor_tensor(out=te, in0=ce, in1=me, op=OP.mult)
                V.tensor_tensor(out=to_, in0=co, in1=mo, op=OP.mult)
                V.tensor_tensor(out=s1_, in0=te, in1=to_, op=OP.add)
                V.tensor_tensor(out=s2, in0=ve, in1=vo, op=OP.add)
                V.tensor_tensor(out=te, in0=te, in1=me, op=OP.mult)
                V.tensor_tensor(out=s2, in0=s2, in1=te, op=OP.add)
                V.tensor_tensor(out=to_, in0=to_, in1=mo, op=OP.mult)
                V.tensor_tensor(out=s2, in0=s2, in1=to_, op=OP.add)
                V.tensor_tensor(out=mean, in0=s1_, in1=invcnt, op=OP.mult)
                V.tensor_tensor(out=s2, in0=s2, in1=invcnt, op=OP.mult)
                V.tensor_tensor(out=var, in0=mean, in1=mean, op=OP.mult)
                V.tensor_tensor(out=var, in0=s2, in1=var, op=OP.subtract)
                V.tensor_scalar(out=var, in0=var, scalar1=0.0,
                                scalar2=BN_EPS, op0=OP.max, op1=OP.add)
                nc.scalar.activation(out=var, in_=var, func=AF.Sqrt)
                V.reciprocal(out=var, in_=var)  # := 1/sqrt(var+eps)
                V.tensor_tensor(out=te, in0=gT_u, in1=var, op=OP.mult)
                V.tensor_tensor(out=a_t[:, sl], in0=te, in1=onem, op=OP.add)
                V.tensor_tensor(out=to_, in0=mean, in1=te, op=OP.mult)
                V.tensor_tensor(out=c_t[:, sl], in0=ubb, in1=to_,
                                op=OP.subtract)
                V.tensor_copy(out=a16[:, 0:s1 - s0], in_=a_t[:, sl])
                V.tensor_copy(out=c16[:, 0:s1 - s0], in_=c_t[:, sl])

            # pooling side: zn transposes -> z_d + edge gathers + D matmuls
            # (gathers fire as soon as their z_d watermark is met; gpsimd
            # is otherwise idle)
            state = {"wc": 0, "et": 0}
            GP = 4

            def edges_upto(final=False):
                while state["et"] < T_E:
                    t0 = state["et"]
                    t1 = t0
                    while (t1 < T_E and t1 - t0 < GP
                           and tile_ready[t1] <= state["wc"]):
                        t1 += 1
                    if t1 == t0 or (t1 - t0 < GP and not final):
                        break
                    wt = t1 - t0
                    zg = bg2.tile([128, GP, DIM], DT16, tag="zg", name="zg",
                                  bufs=2)
                    nc.gpsimd.dma_gather(
                        out_ap=zg[:, :wt, :],
                        in_ap=z_d[0:state["wc"] * 128, :],
                        idxs_ap=egidx_t[:, t0 * 8:t1 * 8],
                        num_idxs=wt * 128, num_idxs_reg=wt * 128,
                        elem_size=DIM, single_packet=True)
                    for t in range(t0, t1):
                        nc.tensor.matmul(dt0[:], lhsT=zg[:, t - t0, :],
                                         rhs=p_sb[:, t, 0:512],
                                         start=(t == 0), stop=(t == T_E - 1))
                        nc.tensor.matmul(dt1[:], lhsT=zg[:, t - t0, :],
                                         rhs=p_sb[:, t, 512:B],
                                         start=(t == 0), stop=(t == T_E - 1))
                    state["et"] = t1

            def pass2_block(b, aT, cT, ns):
                # zn = relu(a_exp*z + c_exp) for block b; transpose chunks
                # -> z rows -> z_d flush -> edge gathers + D matmuls
                base = b * BLK
                a_ex = pep.tile([128, BLK], F32, tag="ex", space="PSUM",
                                name="a_ex")
                c_ex = pep.tile([128, BLK], F32, tag="ex", space="PSUM",
                                name="c_ex")
                nc.tensor.matmul(a_ex[:], lhsT=aT[0:ns, :],
                                 rhs=emat_t[0:ns, base:base + BLK],
                                 start=True, stop=True)
                nc.tensor.matmul(c_ex[:], lhsT=cT[0:ns, :],
                                 rhs=emat_t[0:ns, base:base + BLK],
                                 start=True, stop=True)
                zb = znT[:, base:base + BLK]
                V.tensor_tensor(out=zb, in0=z_sb[:, base:base + BLK],
                                in1=a_ex[:], op=OP.mult)
                V.tensor_tensor(out=zb, in0=zb, in1=c_ex[:], op=OP.add)
                V.tensor_scalar_max(out=zb, in0=zb, scalar1=0.0)
                zr = bg2.tile([128, BCH, DIM], DT16, tag="zr", name="zr",
                              bufs=3)
                for c in range(BCH):
                    ch = base // 128 + c
                    pt = ptp.tile([128, 128], DT16, tag="pt", space="PSUM",
                                  name="pt")
                    nc.tensor.transpose(
                        out=pt[:], in_=znT[:, ch * 128:(ch + 1) * 128],
                        identity=ident16[:])
                    # one engine per block so the z_d flush has a single
                    # precise producer (avoids global-clock fallback waits)
                    if b % 2 == 0:
                        nc.scalar.copy(out=zr[:, c, :], in_=pt[:])
                    else:
                        nc.vector.tensor_copy(out=zr[:, c, :], in_=pt[:])
                nc.sync.dma_start(
                    out=z_d[base:base + BLK, :].rearrange(
                        "(c p) d -> p c d", p=128),
                    in_=zr[:])
                state["wc"] = base // 128 + BCH
                edges_upto(final=(b == NBLK - 1))

            # ---- main schedule -----------------------------------------
            done_g = 0
            done_p1 = 0

            def groups_upto(cn):
                nonlocal done_g
                while done_g < n_gg and done_g * GG < cn:
                    temb_group(done_g)
                    done_g += 1

            def pass1_upto(nb):
                nonlocal done_p1
                while done_p1 < nb:
                    groups_upto(min((done_p1 + 1) * BCH + GG, C_n))
                    pass1_block(done_p1)
                    done_p1 += 1

            for ki, (b0, b1) in enumerate(batches):
                pass1_upto(b1)
                S0 = blk_slots[b0][0]
                S1 = blk_slots[b1 - 1][1]
                ns = S1 - S0
                coefs(S0, S1)
                if ki + 1 < len(batches):
                    # keep PE busy on next batch's pass1 while DVE does coefs
                    pass1_upto(batches[ki + 1][1])
                ap_ = ptp.tile([128, 128], DT16, tag="pt", space="PSUM",
                               name="ap_")
                cp_ = ptp.tile([128, 128], DT16, tag="pt", space="PSUM",
                               name="cp_")
                nc.tensor.transpose(out=ap_[:], in_=a16[:],
                                    identity=ident16[:])
                nc.tensor.transpose(out=cp_[:], in_=c16[:],
                                    identity=ident16[:])
                aT = bg2.tile([128, 128], DT16, tag="aT", name="aT", bufs=2)
                cT = bg2.tile([128, 128], DT16, tag="cT", name="cT", bufs=2)
                nc.scalar.copy(out=aT[:], in_=ap_[:])
                nc.scalar.copy(out=cT[:], in_=cp_[:])
                for b in range(b0, b1):
                    pass2_block(b, aT, cT, ns)

            # ---- AllReduce of D^T partials (relgT/NC folded) ----
            ones = sm.tile([128, 1], DT16)
            nc.vector.memset(ones[:], 1.0)
            dts = bg2.tile([128, B], DT16, tag="dts")
            nc.vector.tensor_tensor(out=dts[:, 0:512], in0=dt0[:],
                                    in1=relgT_t[:, 0:512], op=OP.add)
            nc.vector.tensor_tensor(out=dts[:, 512:B], in0=dt1[:],
                                    in1=relgT_t[:, 512:B], op=OP.add)
            nc.sync.dma_start(out=ard_in[:], in_=dts[:])
            nc.gpsimd.collective_compute(
                "AllReduce", OP.add,
                replica_groups=[list(range(NC))],
                ins=[ard_in.ap().opt()], outs=[ard_out.ap().opt()])

            # ---- scoring ----
            v_t = bg2.tile([128, B], DT16, tag="v_t")
            nc.sync.dma_start(out=v_t[:], in_=ard_out[:])
            sq = bg2.tile([128, B], DT16, tag="sq")
            nc.vector.tensor_tensor(out=sq[:], in0=v_t[:], in1=v_t[:],
                                    op=OP.mult)
            ss0 = ps.tile([1, 512], F32, tag="zp", space="PSUM", name="ss0")
            ss1 = ps.tile([1, 512], F32, tag="zp", space="PSUM", name="ss1")
            nc.tensor.matmul(ss0[:], lhsT=ones[:], rhs=sq[:, 0:512],
                             start=True, stop=True)
            nc.tensor.matmul(ss1[:], lhsT=ones[:], rhs=sq[:, 512:B],
                             start=True, stop=True)
            souts = bg2.tile([1, B], F32, tag="souts")
            nc.scalar.activation(out=souts[:, 0:512], in_=ss0[:],
                                 func=AF.Sqrt)
            nc.scalar.activation(out=souts[:, 512:B], in_=ss1[:],
                                 func=AF.Sqrt)
            nc.vector.tensor_scalar_mul(souts[:], souts[:], -1.0)
            nc.sync.dma_start(out=outd[:], in_=souts[:])

    nc.finalize()
    return nc


_CACHE = {}


def _in_maps(p):
    return [{
        "xrow": p.xrow[c],
        "tcat": p.tcat[c],
        "wloc": p.wloc[c],
        "aux": p.aux[c],
        "emat": p.Emat,
        "egidx": p.egidx[c],
        "p_d": p.P[c],
        "relgT": p.relgT,
        "identd": p.ident,
    } for c in range(NC)]


def kernel(**inputs) -> np.ndarray:
    p = _make_plan(inputs)
    key = (p.SLOTS, tuple(p.caps.tolist()), p.NCOLS, p.N_cap, p.C_n, p.T_E,
           p.tile_ready, p.blk_slots, p.batches)
    if key not in _CACHE:
        _CACHE[key] = _build(p.SLOTS, p.caps, p.col_start, p.N_cap,
                             p.C_n, p.T_E, p.tile_ready, p.blk_slots,
                             p.batches)
    nc = _CACHE[key]
    res = run_bass_kernel_spmd(nc, _in_maps(p), core_ids=list(range(NC)))
    return np.ascontiguousarray(res.results[0]["out"]).reshape(B).astype(
        np.float32)


# revision 53
# speedup vs baseline: 1.1477x; 1.1477x over previous
"""Trainium2 Bass kernel for nn_DE_TGraph (diachronic temporal-graph GNN layer).

Strategy (8 NeuronCores, SPMD, relation-sharded):
  - 460 relations partitioned across 8 cores by size-rank snake order.
    Per-relation BatchNorm is core-local (no collectives for BN stats).
  - Host packs per-neighbor feature rows (ent96 | freq96 | phi96 | amp96)
    in slot-column order per core, so the device loads them with plain
    contiguous DMA (no gpsimd table gather on the critical path).  Slots
    are greedily packed into 512-column blocks (PSUM-bank sized).
  - Time embedding on DVE/ACT (sin on ScalarE) in row layout per
    8-chunk group; x chunks transposed on PE into xT.
  - Pass 1: per-slot GEMM z^T = W_r^T xT into a shared per-block PSUM
    bank + bn_stats per slot; one bf16 copy per block saves z to SBUF
    (no second GEMM pass).
  - BN affine: per batch of blocks, coefficients a,c are computed on
    DVE, transposed on PE, and expanded to per-column vectors via small
    K=|batch slots| matmuls against a host 0/1 expansion matrix;
    zn = relu(a_exp*z + c_exp) via two DVE block ops + one ACT relu
    per 512 block.
  - Per 4-chunk block: PE transposes zn -> z rows (copies rotate over
    ACT/gpsimd/DVE), z_d flushed per block; edge-tile dma_gathers fire
    as soon as their z_d watermark is met (gpsimd is otherwise idle),
    and D^T[d,b] += zg^T @ P_t accumulates in PSUM.  D^T is this core's
    partial of emb[head]-emb[tail]; rel_embs[rels]^T/8 folded in.
  - Warm-up AllReduce at kernel start absorbs CC firmware spin-up; one
    real AllReduce of D^T [128, B] bf16.
  - Scoring: score = -sqrt(sum_d v^2) via square + ones-vector matmul.
"""
import numpy as np

import concourse.bacc as bacc
import concourse.mybir as mybir
import concourse.tile as tile
from concourse import library_config
from concourse.bass_utils import run_bass_kernel_spmd

F32 = mybir.dt.float32
BF16 = mybir.dt.bfloat16
DT16 = BF16
import ml_dtypes
NPDT16 = ml_dtypes.bfloat16
I16 = mybir.dt.int16
AF = mybir.ActivationFunctionType
OP = mybir.AluOpType

NUM_ENT = 10000
NUM_REL = 230
R2 = 2 * NUM_REL
S_DIM = 96
T_DIM = 32
DIM = 128
N = 32768
Q = 4096
E = 32768
B = 1024
NC = 8
BN_EPS = 1e-5
BLK = 512          # PSUM-bank block (columns)
BCH = BLK // 128   # chunks per block (4)
NBATCH = 3         # coef batches

# Probed XOR-peer map: BETA[a][d] = logical rank of core a's remote_dma
# peer at relative Δtpb=d (hardware XORs *physical* tpb; the driver's
# logical->physical NC map is identity on die 0 and ^2 on die 1).
# BETA[a][j] is also the global triple block held at core a's local
# block j in the butterfly exchange.
BETA = (
    (0, 1, 2, 3, 6, 7, 4, 5),
    (1, 0, 3, 2, 7, 6, 5, 4),
    (2, 3, 0, 1, 4, 5, 6, 7),
    (3, 2, 1, 0, 5, 4, 7, 6),
    (4, 5, 6, 7, 2, 3, 0, 1),
    (5, 4, 7, 6, 3, 2, 1, 0),
    (6, 7, 4, 5, 0, 1, 2, 3),
    (7, 6, 5, 4, 1, 0, 3, 2),
)


def _wrap16(idx, n):
    """[n] int -> [128, n//16] int16 (index i at [i%16, i//16], replicated)."""
    assert n % 16 == 0 and len(idx) == n
    w = np.asarray(idx).reshape(n // 16, 16).T.astype(np.int16)
    return np.ascontiguousarray(np.tile(w, (8, 1)))


def _tile_rows(x, p=128):
    """[n, d] -> [128, n//p, d] tile layout (row c*128+p -> [p, c, :])."""
    n, d = x.shape
    assert n % p == 0
    return np.ascontiguousarray(x.reshape(n // p, p, d).transpose(1, 0, 2))


class _Plan:
    pass


def _make_plan(inp):
    p = _Plan()
    rel = np.asarray(inp["rel_id"]).astype(np.int64).reshape(-1)
    nidx = np.asarray(inp["neighbor_idx"]).astype(np.int64).reshape(-1)
    years = np.asarray(inp["years"], np.float32).reshape(-1)
    months = np.asarray(inp["months"], np.float32).reshape(-1)
    days = np.asarray(inp["days"], np.float32).reshape(-1)
    psrc = np.asarray(inp["pool_src"]).astype(np.int64).reshape(-1)
    pdst = np.asarray(inp["pool_dst"]).astype(np.int64).reshape(-1)
    head = np.asarray(inp["head_pos"]).astype(np.int64).reshape(-1)
    tail = np.asarray(inp["tail_pos"]).astype(np.int64).reshape(-1)
    rels = np.asarray(inp["rels"]).astype(np.int64).reshape(-1)
    W = np.asarray(inp["W"], np.float32)
    b = np.asarray(inp["b"], np.float32)
    gamma = np.asarray(inp["gamma"], np.float32)
    beta = np.asarray(inp["beta"], np.float32)

    # ---- relation partition: size-rank snake across cores ----
    cnts = np.bincount(rel, minlength=R2)
    order = np.argsort(-cnts, kind="stable")
    SLOTS = (R2 + NC - 1) // NC
    core_rels = [[None] * SLOTS for _ in range(NC)]
    caps = np.zeros(SLOTS, np.int64)
    for s in range(SLOTS):
        grp = order[s * NC : (s + 1) * NC]
        caps[s] = cnts[grp[0]]  # exact group max; only block tails padded
        perm = range(NC) if s % 2 == 0 else range(NC - 1, -1, -1)
        for r, c in zip(grp, perm):
            core_rels[c][s] = int(r)
    assert caps.max() <= BLK, "relation bucket exceeds 512 (unsupported)"

    # greedy-pack slots into 512-col blocks; each block padded to 512
    col_start = np.zeros(SLOTS + 1, np.int64)
    blk_slots = []  # (s0, s1) slot range per block
    cur = 0
    s0 = 0
    for s in range(SLOTS):
        if cur % BLK + caps[s] > BLK:
            blk_slots.append((s0, s))
            s0 = s
            cur = (cur // BLK + 1) * BLK
        col_start[s] = cur
        cur += int(caps[s])
    blk_slots.append((s0, SLOTS))
    col_start[SLOTS] = cur
    NCOLS = int(cur)
    NBLK = len(blk_slots)
    N_cap = NBLK * BLK
    C_n = N_cap // 128

    # coef batches over block ranges (host + device must agree)
    bper = (NBLK + NBATCH - 1) // NBATCH
    batches = tuple((k * bper, min((k + 1) * bper, NBLK))
                    for k in range(NBATCH) if k * bper < NBLK)

    # positions per relation
    order_by_rel = np.argsort(rel, kind="stable")
    rel_sorted = rel[order_by_rel]
    starts = np.searchsorted(rel_sorted, np.arange(R2))
    ends = np.searchsorted(rel_sorted, np.arange(R2), side="right")

    pos_core = np.empty(N, np.int64)
    pos_col = np.empty(N, np.int64)

    # packed per-entity table (bf16): ent96 | yf mf df | yp mp dp | ya ma da
    tbl = np.zeros((NUM_ENT, 384), np.float32)
    tbl[:, 0:96] = np.asarray(inp["ent_embs"], np.float32)
    for k, nm in enumerate(["y_freq", "m_freq", "d_freq",
                            "y_phi", "m_phi", "d_phi",
                            "y_amp", "m_amp", "d_amp"]):
        tbl[:, 96 + 32 * k : 128 + 32 * k] = np.asarray(inp[nm], np.float32)
    tbl16 = tbl.astype(NPDT16)

    xrow_cores, tcat_cores = [], []
    for c in range(NC):
        gidx = np.zeros(N_cap, np.int64)
        valid = np.zeros(N_cap, bool)
        tc3 = np.zeros((N_cap, 3), np.float32)
        for s in range(SLOTS):
            r = core_rels[c][s]
            if r is None:
                continue
            pp = order_by_rel[starts[r] : ends[r]]
            pp = pp[np.argsort(nidx[pp], kind="stable")]
            j0 = int(col_start[s])
            gidx[j0 : j0 + len(pp)] = nidx[pp]
            valid[j0 : j0 + len(pp)] = True
            tc3[j0 : j0 + len(pp), 0] = years[pp]
            tc3[j0 : j0 + len(pp), 1] = months[pp]
            tc3[j0 : j0 + len(pp), 2] = days[pp]
            pos_core[pp] = c
            pos_col[pp] = j0 + np.arange(len(pp))
        xr = np.zeros((N_cap, 384), NPDT16)
        xr[valid] = tbl16[gidx[valid]]
        xrow_cores.append(_tile_rows(xr))
        tcat_cores.append(_tile_rows(tc3).astype(NPDT16))

    # ---- batch-local 0/1 expansion matrix [64, N_cap] ----
    Emat = np.zeros((64, N_cap), np.float32)
    for (b0, b1) in batches:
        S0 = blk_slots[b0][0]
        S1 = blk_slots[b1 - 1][1]
        assert S1 - S0 <= 64
        for s in range(S0, S1):
            Emat[s - S0, int(col_start[s]) : int(col_start[s]) + int(caps[s])] = 1.0
    p.Emat = np.ascontiguousarray(Emat.astype(NPDT16))

    # ---- per-core weights + BN aux ----
    wloc_cores, aux_cores = [], []
    for c in range(NC):
        wl = np.zeros((SLOTS, DIM, DIM), np.float32)
        invcnt = np.ones(SLOTS, np.float32)
        onem = np.ones(SLOTS, np.float32)
        gT_u = np.zeros((DIM, SLOTS), np.float32)
        ubb = np.zeros((DIM, SLOTS), np.float32)
        for s in range(SLOTS):
            r = core_rels[c][s]
            if r is None:
                continue
            wl[s] = W[r]
            cnt = cnts[r]
            invcnt[s] = 1.0 / max(cnt, 1)
            u = 1.0 if cnt > 1 else 0.0
            onem[s] = 1.0 - u
            gT_u[:, s] = gamma[r] * u
            ubb[:, s] = beta[r] * u + b[r] * (1.0 - u)
        aux = np.zeros((128, 4 * SLOTS), np.float32)
        aux[:, 0:SLOTS] = invcnt[None, :]
        aux[:, SLOTS : 2 * SLOTS] = onem[None, :]
        aux[:, 2 * SLOTS : 3 * SLOTS] = gT_u
        aux[:, 3 * SLOTS : 4 * SLOTS] = ubb
        wloc_cores.append(np.ascontiguousarray(
            wl.transpose(1, 0, 2).astype(NPDT16)))
        aux_cores.append(np.ascontiguousarray(aux))

    # ---- pooling edges: keep only dsts referenced by head/tail ----
    pcnt = np.bincount(pdst, minlength=Q).astype(np.float32)
    used = np.zeros(Q, bool)
    used[head] = True
    used[tail] = True
    keep = used[pdst]
    e_core = pos_core[psrc]
    ecols, edsts = [], []
    for c in range(NC):
        m = keep & (e_core == c)
        es, ed = psrc[m], pdst[m]
        o = np.argsort(pos_col[es], kind="stable")  # z_d locality + watermark
        ecols.append(pos_col[es[o]])
        edsts.append(ed[o])

    # dedup edge srcs: one gathered z row per distinct src, P rows summed
    uniq_cores = [np.unique(x) for x in ecols]
    T_E = max(1, max((len(u) + 127) // 128 for u in uniq_cores))
    NE = T_E * 128

    egidx_cores, p_cores = [], []
    tile_ready = np.zeros(T_E, np.int64)  # z_d chunks needed per edge tile
    for c in range(NC):
        ec, ed = ecols[c], edsts[c]
        uniq = uniq_cores[c]
        rows = np.searchsorted(uniq, ec)
        inv = 1.0 / np.maximum(pcnt[ed], 1.0)
        contrib = ((ed[:, None] == head[None, :]).astype(np.float32)
                   - (ed[:, None] == tail[None, :]).astype(np.float32))
        contrib *= inv[:, None]
        P = np.zeros((NE, B), np.float32)
        np.add.at(P, rows, contrib)
        eg = np.zeros(NE, np.int64)
        eg[: len(uniq)] = uniq
        egidx_cores.append(_wrap16(eg, NE))
        p_cores.append(np.ascontiguousarray(
            P.reshape(T_E, 128, B).transpose(1, 0, 2).astype(NPDT16)))
        hi = eg.reshape(T_E, 128).max(axis=1)  # cols sorted -> per-tile max
        tile_ready = np.maximum(tile_ready, hi // 128 + 1)
    p.tile_ready = tuple(int(x) for x in tile_ready)

    # ---- scoring: rel_embs[rels]^T / NC, folded pre-reduce ----
    # Butterfly exchange uses XOR-local triple blocks: on core c, local
    # 128-col block j holds global block c ^ j.  Permute P columns and
    # relgT per core accordingly; core 0's local order is then global.
    relgT = np.asarray(inp["rel_embs"], np.float32)[rels].T / NC  # [128, B]
    p.relgT = np.ascontiguousarray(relgT.astype(NPDT16))
    p.ident = np.ascontiguousarray(np.eye(128, dtype=NPDT16))

    p.SLOTS, p.caps, p.col_start = SLOTS, caps, col_start
    p.NCOLS, p.N_cap, p.C_n, p.T_E = NCOLS, N_cap, C_n, T_E
    p.NBLK, p.blk_slots, p.batches = NBLK, tuple(blk_slots), batches
    p.xrow, p.tcat = xrow_cores, tcat_cores
    p.wloc, p.aux = wloc_cores, aux_cores
    p.egidx, p.P = egidx_cores, p_cores
    return p


def _build(SLOTS, caps, col_start, N_cap, C_n, T_E, tile_ready, blk_slots,
           batches):
    NBLK = len(blk_slots)
    nc = bacc.Bacc(None, target_bir_lowering=False, debug=False,
                   num_devices=NC, num_swdge_queues=2)
    xrow = nc.dram_tensor("xrow", [128, C_n, 384], DT16, kind="ExternalInput")
    tcat = nc.dram_tensor("tcat", [128, C_n, 3], DT16, kind="ExternalInput")
    wloc = nc.dram_tensor("wloc", [128, SLOTS, DIM], DT16,
                          kind="ExternalInput")
    aux = nc.dram_tensor("aux", [128, 4 * SLOTS], F32, kind="ExternalInput")
    emat = nc.dram_tensor("emat", [64, N_cap], DT16, kind="ExternalInput")
    egidx = nc.dram_tensor("egidx", [128, T_E * 128 // 16], I16,
                           kind="ExternalInput")
    p_d = nc.dram_tensor("p_d", [128, T_E, B], DT16, kind="ExternalInput")
    relgT = nc.dram_tensor("relgT", [128, B], DT16, kind="ExternalInput")
    identd = nc.dram_tensor("identd", [128, 128], DT16, kind="ExternalInput")
    outd = nc.dram_tensor("out", [1, B], F32, kind="ExternalOutput")

    z_d = nc.dram_tensor("z_d", [N_cap, DIM], DT16)
    ard_in = nc.dram_tensor("ard_in", [128, B], DT16)
    ard_out = nc.dram_tensor("ard_out", [128, B], DT16, addr_space="Shared")
    war_in = nc.dram_tensor("war_in", [128, 16], DT16)
    war_out = nc.dram_tensor("war_out", [128, 16], DT16, addr_space="Shared")

    GG = 8  # chunks per x-load / time-embedding group
    n_gg = (C_n + GG - 1) // GG


    with tile.TileContext(nc) as tc:
        with (
            tc.tile_pool(name="pers", bufs=1) as sm,
            tc.tile_pool(name="ps", bufs=2, space="PSUM") as ps,
            tc.tile_pool(name="ptp", bufs=2, space="PSUM") as ptp,
            tc.tile_pool(name="pep", bufs=2, space="PSUM") as pep,
            tc.tile_pool(name="dtp", bufs=1, space="PSUM") as dtp,
            tc.tile_pool(name="ph2", bufs=1) as bg2,
        ):
            nc.gpsimd.load_library(library_config.mlp)

            def load(pool, name, dram, shape, dtype=F32):
                t = pool.tile(shape, dtype, tag=name, name=name)
                nc.sync.dma_start(out=t[:], in_=dram[:])
                return t

            # warm-up collective: pays the CC firmware spin-up cost and
            # re-synchronizes the cores early, off the critical path
            wu = sm.tile([128, 16], DT16, tag="wu")
            nc.vector.memset(wu[:], 0.0)
            nc.sync.dma_start(out=war_in[:], in_=wu[:])
            nc.gpsimd.collective_compute(
                "AllReduce", OP.add,
                replica_groups=[list(range(NC))],
                ins=[war_in.ap().opt()], outs=[war_out.ap().opt()])
            # x rows arrive per 8-chunk group (first-needed-first DMA order)
            xr_t = sm.tile([128, C_n, 384], DT16, tag="xr")
            for g in range(n_gg):
                c0, c1 = g * GG, min((g + 1) * GG, C_n)
                nc.sync.dma_start(out=xr_t[:, c0:c1, :],
                                  in_=xrow[:, c0:c1, :])
            tcat_t = load(sm, "tcat", tcat, [128, C_n, 3], DT16)
            ident16 = load(sm, "identd", identd, [128, 128], DT16)
            w_sb = load(sm, "w_sb", wloc, [128, SLOTS, DIM], DT16)
            aux_t = load(sm, "aux", aux, [128, 4 * SLOTS])
            emat_t = load(sm, "emat", emat, [64, N_cap], DT16)
            egidx_t = load(sm, "egidx", egidx, [128, T_E * 128 // 16], I16)
            p_sb = load(sm, "p_sb", p_d, [128, T_E, B], DT16)
            relgT_t = load(sm, "relgT", relgT, [128, B], DT16)

            xT = sm.tile([128, N_cap], DT16)
            z_sb = sm.tile([128, N_cap], DT16)
            znT = sm.tile([128, N_cap], DT16)
            stats6 = sm.tile([128, SLOTS, 6], F32)
            a_t = sm.tile([128, SLOTS], F32, tag="a_t")
            c_t = sm.tile([128, SLOTS], F32, tag="c_t")
            a16 = sm.tile([128, 128], DT16, tag="a16")
            c16 = sm.tile([128, 128], DT16, tag="c16")
            nc.vector.memset(a16[:], 0.0)
            nc.vector.memset(c16[:], 0.0)
            sc = [sm.tile([128, SLOTS], F32, tag=f"sc{i}", name=f"sc{i}")
                  for i in range(6)]
            V = nc.vector

            dt0 = dtp.tile([128, 512], F32, tag="dt0", space="PSUM",
                           name="dt0")
            dt1 = dtp.tile([128, 512], F32, tag="dt1", space="PSUM",
                           name="dt1")

            # ---- helpers ------------------------------------------------
            cp_eng = [0]

            def next_copy(out, in_):
                # rotate PSUM->SBUF copies over ACT / DVE (gpsimd can't
                # read PSUM)
                k = cp_eng[0] % 2
                cp_eng[0] += 1
                if k == 0:
                    nc.scalar.copy(out=out, in_=in_)
                else:
                    nc.vector.tensor_copy(out=out, in_=in_)

            def temb_group(g):
                # time embedding for chunks [c0,c1): x[:, :, 96:128] final
                c0, c1 = g * GG, min((g + 1) * GG, C_n)
                w = c1 - c0
                gb = xr_t[:, c0:c1, :]
                xs = bg2.tile([128, GG, 96], DT16, tag="xs", name="xs",
                              bufs=2)
                f4 = gb[:, :, 96:192].rearrange("p c (k e) -> p c k e", k=3)
                x4 = xs[:, :w, :].rearrange("p c (k e) -> p c k e", k=3)
                t4 = tcat_t[:, c0:c1, :].unsqueeze(3).to_broadcast(
                    [128, w, 3, T_DIM])
                V.tensor_tensor(out=x4, in0=f4, in1=t4, op=OP.mult)
                V.tensor_tensor(out=xs[:, :w, :], in0=xs[:, :w, :],
                                in1=gb[:, :, 192:288], op=OP.add)
                nc.scalar.activation(out=xs[:, :w, :], in_=xs[:, :w, :],
                                     func=AF.Sin)
                V.tensor_tensor(out=xs[:, :w, :], in0=xs[:, :w, :],
                                in1=gb[:, :, 288:384], op=OP.mult)
                V.tensor_tensor(out=gb[:, :, 96:128], in0=xs[:, :w, 0:32],
                                in1=xs[:, :w, 32:64], op=OP.add)
                V.tensor_tensor(out=gb[:, :, 96:128], in0=gb[:, :, 96:128],
                                in1=xs[:, :w, 64:96], op=OP.add)
                for c in range(c0, c1):
                    pt = ptp.tile([128, 128], DT16, tag="pt", space="PSUM",
                                  name="pt")
                    nc.tensor.transpose(out=pt[:], in_=gb[:, c - c0, 0:128],
                                        identity=ident16[:])
                    next_copy(xT[:, c * 128:(c + 1) * 128], pt[:])

            def pass1_block(b):
                # per-slot GEMM into one shared PSUM bank + stats + z save
                s0, s1 = blk_slots[b]
                base = b * BLK
                zp = ps.tile([128, BLK], F32, tag="zp", space="PSUM",
                             name="zp")
                for s in range(s0, s1):
                    a = int(col_start[s])
                    bb = a + int(caps[s])
                    if s == s1 - 1:
                        bb = base + BLK  # cover block pad (xT cols are zero)
                    nc.tensor.matmul(zp[:, a - base:bb - base],
                                     lhsT=w_sb[:, s, :], rhs=xT[:, a:bb],
                                     start=True, stop=True)
                for s in range(s0, s1):
                    a = int(col_start[s])
                    bb = a + int(caps[s])
                    nc.vector.bn_stats(stats6[:, s, :],
                                       zp[:, a - base:bb - base])
                V.tensor_copy(out=z_sb[:, base:base + BLK], in_=zp[:])

            def coefs(s0, s1):
                # BN coefs a,c for slots [s0,s1) + batch-local bf16 pack
                sl = slice(s0, s1)
                ce, me, ve = (stats6[:, sl, k] for k in (0, 1, 2))
                co, mo, vo = (stats6[:, sl, k] for k in (3, 4, 5))
                invcnt = aux_t[:, s0:s1]
                onem = aux_t[:, SLOTS + s0:SLOTS + s1]
                gT_u = aux_t[:, 2 * SLOTS + s0:2 * SLOTS + s1]
                ubb = aux_t[:, 3 * SLOTS + s0:3 * SLOTS + s1]
                te, to_, s1_, s2, mean, var = (t[:, sl] for t in sc)
                V.tensor_tensor(out=te, in0=ce, in1=me, op=OP.mult)
                V.tensor_tensor(out=to_, in0=co, in1=mo, op=OP.mult)
                V.tensor_tensor(out=s1_, in0=te, in1=to_, op=OP.add)
                V.tensor_tensor(out=s2, in0=ve, in1=vo, op=OP.add)
                V.tensor_tensor(out=te, in0=te, in1=me, op=OP.mult)
                V.tensor_tensor(out=s2, in0=s2, in1=te, op=OP.add)
                V.tensor_tensor(out=to_, in0=to_, in1=mo, op=OP.mult)
                V.tensor_tensor(out=s2, in0=s2, in1=to_, op=OP.add)
                V.tensor_tensor(out=mean, in0=s1_, in1=invcnt, op=OP.mult)
                V.tensor_tensor(out=s2, in0=s2, in1=invcnt, op=OP.mult)
                V.tensor_tensor(out=var, in0=mean, in1=mean, op=OP.mult)
                V.tensor_tensor(out=var, in0=s2, in1=var, op=OP.subtract)
                V.tensor_scalar(out=var, in0=var, scalar1=0.0,
                                scalar2=BN_EPS, op0=OP.max, op1=OP.add)
                nc.scalar.activation(out=var, in_=var, func=AF.Sqrt)
                V.reciprocal(out=var, in_=var)  # := 1/sqrt(var+eps)
                V.tensor_tensor(out=te, in0=gT_u, in1=var, op=OP.mult)
                V.tensor_tensor(out=a_t[:, sl], in0=te, in1=onem, op=OP.add)
                V.tensor_tensor(out=to_, in0=mean, in1=te, op=OP.mult)
                V.tensor_tensor(out=c_t[:, sl], in0=ubb, in1=to_,
                                op=OP.subtract)
                V.tensor_copy(out=a16[:, 0:s1 - s0], in_=a_t[:, sl])
                V.tensor_copy(out=c16[:, 0:s1 - s0], in_=c_t[:, sl])

            # pooling side: zn transposes -> z_d + edge gathers + D matmuls
            # (gathers fire as soon as their z_d watermark is met; gpsimd
            # is otherwise idle)
            state = {"wc": 0, "et": 0}
            GP = 4

            def edges_upto(final=False):
                while state["et"] < T_E:
                    t0 = state["et"]
                    t1 = t0
                    while (t1 < T_E and t1 - t0 < GP
                           and tile_ready[t1] <= state["wc"]):
                        t1 += 1
                    if t1 == t0 or (t1 - t0 < GP and not final):
                        break
                    wt = t1 - t0
                    zg = bg2.tile([128, GP, DIM], DT16, tag="zg", name="zg",
                                  bufs=2)
                    nc.gpsimd.dma_gather(
                        out_ap=zg[:, :wt, :],
                        in_ap=z_d[0:state["wc"] * 128, :],
                        idxs_ap=egidx_t[:, t0 * 8:t1 * 8],
                        num_idxs=wt * 128, num_idxs_reg=wt * 128,
                        elem_size=DIM, single_packet=True)
                    for t in range(t0, t1):
                        nc.tensor.matmul(dt0[:], lhsT=zg[:, t - t0, :],
                                         rhs=p_sb[:, t, 0:512],
                                         start=(t == 0), stop=(t == T_E - 1))
                        nc.tensor.matmul(dt1[:], lhsT=zg[:, t - t0, :],
                                         rhs=p_sb[:, t, 512:B],
                                         start=(t == 0), stop=(t == T_E - 1))
                    state["et"] = t1

            def pass2_block(b, aT, cT, ns):
                # zn = relu(a_exp*z + c_exp) for block b; transpose chunks
                # -> z rows -> z_d flush -> edge gathers + D matmuls
                base = b * BLK
                a_ex = pep.tile([128, BLK], F32, tag="ex", space="PSUM",
                                name="a_ex")
                c_ex = pep.tile([128, BLK], F32, tag="ex", space="PSUM",
                                name="c_ex")
                nc.tensor.matmul(a_ex[:], lhsT=aT[0:ns, :],
                                 rhs=emat_t[0:ns, base:base + BLK],
                                 start=True, stop=True)
                nc.tensor.matmul(c_ex[:], lhsT=cT[0:ns, :],
                                 rhs=emat_t[0:ns, base:base + BLK],
                                 start=True, stop=True)
                zb = znT[:, base:base + BLK]
                V.tensor_tensor(out=zb, in0=z_sb[:, base:base + BLK],
                                in1=a_ex[:], op=OP.mult)
                V.tensor_tensor(out=zb, in0=zb, in1=c_ex[:], op=OP.add)
                V.tensor_scalar_max(out=zb, in0=zb, scalar1=0.0)
                zr = bg2.tile([128, BCH, DIM], DT16, tag="zr", name="zr",
                              bufs=3)
                for c in range(BCH):
                    ch = base // 128 + c
                    pt = ptp.tile([128, 128], DT16, tag="pt", space="PSUM",
                                  name="pt")
                    nc.tensor.transpose(
                        out=pt[:], in_=znT[:, ch * 128:(ch + 1) * 128],
                        identity=ident16[:])
                    # one engine per block so the z_d flush has a single
                    # precise producer (avoids global-clock fallback waits)
                    if b % 2 == 0:
                        nc.scalar.copy(out=zr[:, c, :], in_=pt[:])
                    else:
                        nc.vector.tensor_copy(out=zr[:, c, :], in_=pt[:])
                nc.sync.dma_start(
                    out=z_d[base:base + BLK, :].rearrange(
                        "(c p) d -> p c d", p=128),
                    in_=zr[:])
                state["wc"] = base // 128 + BCH
                edges_upto(final=(b == NBLK - 1))

            # ---- main schedule -----------------------------------------
            done_g = 0
            done_p1 = 0

            def groups_upto(cn):
                nonlocal done_g
                while done_g < n_gg and done_g * GG < cn:
                    temb_group(done_g)
                    done_g += 1

            def pass1_upto(nb):
                nonlocal done_p1
                while done_p1 < nb:
                    groups_upto(min((done_p1 + 1) * BCH + GG, C_n))
                    pass1_block(done_p1)
                    done_p1 += 1

            # deferred-pass2 pipeline: batch k's coefs run on DVE while PE
            # executes pass2 of batch k-1 (kills the PE bubble at each
            # coef-batch boundary)
            pend = None
            for ki, (b0, b1) in enumerate(batches):
                pass1_upto(b1)
                S0 = blk_slots[b0][0]
                S1 = blk_slots[b1 - 1][1]
                ns = S1 - S0
                coefs(S0, S1)
                if ki + 1 < len(batches):
                    # keep PE busy on next batch's pass1 while DVE does coefs
                    pass1_upto(batches[ki + 1][1])
                if pend is not None:
                    for b in range(pend[0], pend[1]):
                        pass2_block(b, pend[2], pend[3], pend[4])
                ap_ = ptp.tile([128, 128], DT16, tag="pt", space="PSUM",
                               name="ap_")
                cp_ = ptp.tile([128, 128], DT16, tag="pt", space="PSUM",
                               name="cp_")
                nc.tensor.transpose(out=ap_[:], in_=a16[:],
                                    identity=ident16[:])
                nc.tensor.transpose(out=cp_[:], in_=c16[:],
                                    identity=ident16[:])
                aT = bg2.tile([128, 128], DT16, tag="aT", name="aT", bufs=2)
                cT = bg2.tile([128, 128], DT16, tag="cT", name="cT", bufs=2)
                nc.scalar.copy(out=aT[:], in_=ap_[:])
                nc.scalar.copy(out=cT[:], in_=cp_[:])
                pend = (b0, b1, aT, cT, ns)
            for b in range(pend[0], pend[1]):
                pass2_block(b, pend[2], pend[3], pend[4])

            # ---- AllReduce of D^T partials (relgT/NC folded) ----
            ones = sm.tile([128, 1], DT16)
            nc.vector.memset(ones[:], 1.0)
            dts = bg2.tile([128, B], DT16, tag="dts")
            nc.vector.tensor_tensor(out=dts[:, 0:512], in0=dt0[:],
                                    in1=relgT_t[:, 0:512], op=OP.add)
            nc.vector.tensor_tensor(out=dts[:, 512:B], in0=dt1[:],
                                    in1=relgT_t[:, 512:B], op=OP.add)
            nc.sync.dma_start(out=ard_in[:], in_=dts[:])
            nc.gpsimd.collective_compute(
                "AllReduce", OP.add,
                replica_groups=[list(range(NC))],
                ins=[ard_in.ap().opt()], outs=[ard_out.ap().opt()])

            # ---- scoring ----
            v_t = bg2.tile([128, B], DT16, tag="v_t")
            nc.sync.dma_start(out=v_t[:], in_=ard_out[:])
            sq = bg2.tile([128, B], DT16, tag="sq")
            nc.vector.tensor_tensor(out=sq[:], in0=v_t[:], in1=v_t[:],
                                    op=OP.mult)
            ss0 = ps.tile([1, 512], F32, tag="zp", space="PSUM", name="ss0")
            ss1 = ps.tile([1, 512], F32, tag="zp", space="PSUM", name="ss1")
            nc.tensor.matmul(ss0[:], lhsT=ones[:], rhs=sq[:, 0:512],
                             start=True, stop=True)
            nc.tensor.matmul(ss1[:], lhsT=ones[:], rhs=sq[:, 512:B],
                             start=True, stop=True)
            souts = bg2.tile([1, B], F32, tag="souts")
            nc.scalar.activation(out=souts[:, 0:512], in_=ss0[:],
                                 func=AF.Sqrt)
            nc.scalar.activation(out=souts[:, 512:B], in_=ss1[:],
                                 func=AF.Sqrt)
            nc.vector.tensor_scalar_mul(souts[:], souts[:], -1.0)
            nc.sync.dma_start(out=outd[:], in_=souts[:])

    nc.finalize()
    return nc


_CACHE = {}


def _in_maps(p):
    return [{
        "xrow": p.xrow[c],
        "tcat": p.tcat[c],
        "wloc": p.wloc[c],
        "aux": p.aux[c],
        "emat": p.Emat,
        "egidx": p.egidx[c],
        "p_d": p.P[c],
        "relgT": p.relgT,
        "identd": p.ident,
    } for c in range(NC)]


def kernel(**inputs) -> np.ndarray:
    p = _make_plan(inputs)
    key = (p.SLOTS, tuple(p.caps.tolist()), p.NCOLS, p.N_cap, p.C_n, p.T_E,
           p.tile_ready, p.blk_slots, p.batches)
    if key not in _CACHE:
        _CACHE[key] = _build(p.SLOTS, p.caps, p.col_start, p.N_cap,
                             p.C_n, p.T_E, p.tile_ready, p.blk_slots,
                             p.batches)
    nc = _CACHE[key]
    res = run_bass_kernel_spmd(nc, _in_maps(p), core_ids=list(range(NC)))
    return np.ascontiguousarray(res.results[0]["out"]).reshape(B).astype(
        np.float32)


# revision 54
# speedup vs baseline: 1.2893x; 1.1234x over previous
"""Trainium2 Bass kernel for nn_DE_TGraph (diachronic temporal-graph GNN layer).

Strategy (8 NeuronCores, SPMD, relation-sharded):
  - 460 relations partitioned across 8 cores by size-rank snake order.
    Per-relation BatchNorm is core-local (no collectives for BN stats).
  - Host packs per-neighbor feature rows (ent96 | freq96 | phi96 | amp96)
    in slot-column order per core, so the device loads them with plain
    contiguous DMA (no gpsimd table gather on the critical path).  Slots
    are greedily packed into 512-column blocks (PSUM-bank sized).
  - Time embedding on DVE/ACT (sin on ScalarE) in row layout per
    8-chunk group; x chunks transposed on PE into xT.
  - Pass 1: per-slot GEMM z^T = W_r^T xT into a shared per-block PSUM
    bank + bn_stats per slot; one bf16 copy per block saves z to SBUF
    (no second GEMM pass).
  - BN affine: per batch of blocks, coefficients a,c are computed on
    DVE, transposed on PE, and expanded to per-column vectors via small
    K=|batch slots| matmuls against a host 0/1 expansion matrix;
    zn = relu(a_exp*z + c_exp) via two DVE block ops + one ACT relu
    per 512 block.
  - Per 4-chunk block: PE transposes zn -> z rows (copies rotate over
    ACT/gpsimd/DVE), z_d flushed per block; edge-tile dma_gathers fire
    as soon as their z_d watermark is met (gpsimd is otherwise idle),
    and D^T[d,b] += zg^T @ P_t accumulates in PSUM.  D^T is this core's
    partial of emb[head]-emb[tail]; rel_embs[rels]^T/8 folded in.
  - Warm-up AllReduce at kernel start absorbs CC firmware spin-up; one
    real AllReduce of D^T [128, B] bf16.
  - Scoring: score = -sqrt(sum_d v^2) via square + ones-vector matmul.
"""
import numpy as np

import concourse.bacc as bacc
import concourse.mybir as mybir
import concourse.tile as tile
from concourse import library_config
from concourse.bass_utils import run_bass_kernel_spmd

F32 = mybir.dt.float32
BF16 = mybir.dt.bfloat16
DT16 = BF16
import ml_dtypes
NPDT16 = ml_dtypes.bfloat16
I16 = mybir.dt.int16
AF = mybir.ActivationFunctionType
OP = mybir.AluOpType

NUM_ENT = 10000
NUM_REL = 230
R2 = 2 * NUM_REL
S_DIM = 96
T_DIM = 32
DIM = 128
N = 32768
Q = 4096
E = 32768
B = 1024
NC = 8
BN_EPS = 1e-5
BLK = 512          # PSUM-bank block (columns)
BCH = BLK // 128   # chunks per block (4)
NBATCH = 3         # coef batches

# Probed XOR-peer map: BETA[a][d] = logical rank of core a's remote_dma
# peer at relative Δtpb=d (hardware XORs *physical* tpb; the driver's
# logical->physical NC map is identity on die 0 and ^2 on die 1).
# BETA[a][j] is also the global triple block held at core a's local
# block j in the butterfly exchange.
BETA = (
    (0, 1, 2, 3, 6, 7, 4, 5),
    (1, 0, 3, 2, 7, 6, 5, 4),
    (2, 3, 0, 1, 4, 5, 6, 7),
    (3, 2, 1, 0, 5, 4, 7, 6),
    (4, 5, 6, 7, 2, 3, 0, 1),
    (5, 4, 7, 6, 3, 2, 1, 0),
    (6, 7, 4, 5, 0, 1, 2, 3),
    (7, 6, 5, 4, 1, 0, 3, 2),
)


def _wrap16(idx, n):
    """[n] int -> [128, n//16] int16 (index i at [i%16, i//16], replicated)."""
    assert n % 16 == 0 and len(idx) == n
    w = np.asarray(idx).reshape(n // 16, 16).T.astype(np.int16)
    return np.ascontiguousarray(np.tile(w, (8, 1)))


def _tile_rows(x, p=128):
    """[n, d] -> [128, n//p, d] tile layout (row c*128+p -> [p, c, :])."""
    n, d = x.shape
    assert n % p == 0
    return np.ascontiguousarray(x.reshape(n // p, p, d).transpose(1, 0, 2))


class _Plan:
    pass


def _make_plan(inp):
    p = _Plan()
    rel = np.asarray(inp["rel_id"]).astype(np.int64).reshape(-1)
    nidx = np.asarray(inp["neighbor_idx"]).astype(np.int64).reshape(-1)
    years = np.asarray(inp["years"], np.float32).reshape(-1)
    months = np.asarray(inp["months"], np.float32).reshape(-1)
    days = np.asarray(inp["days"], np.float32).reshape(-1)
    psrc = np.asarray(inp["pool_src"]).astype(np.int64).reshape(-1)
    pdst = np.asarray(inp["pool_dst"]).astype(np.int64).reshape(-1)
    head = np.asarray(inp["head_pos"]).astype(np.int64).reshape(-1)
    tail = np.asarray(inp["tail_pos"]).astype(np.int64).reshape(-1)
    rels = np.asarray(inp["rels"]).astype(np.int64).reshape(-1)
    W = np.asarray(inp["W"], np.float32)
    b = np.asarray(inp["b"], np.float32)
    gamma = np.asarray(inp["gamma"], np.float32)
    beta = np.asarray(inp["beta"], np.float32)

    # ---- relation partition: size-rank snake across cores ----
    cnts = np.bincount(rel, minlength=R2)
    order = np.argsort(-cnts, kind="stable")
    SLOTS = (R2 + NC - 1) // NC
    core_rels = [[None] * SLOTS for _ in range(NC)]
    caps = np.zeros(SLOTS, np.int64)
    for s in range(SLOTS):
        grp = order[s * NC : (s + 1) * NC]
        caps[s] = cnts[grp[0]]  # exact group max; only block tails padded
        perm = range(NC) if s % 2 == 0 else range(NC - 1, -1, -1)
        for r, c in zip(grp, perm):
            core_rels[c][s] = int(r)
    assert caps.max() <= BLK, "relation bucket exceeds 512 (unsupported)"

    # greedy-pack slots into 512-col blocks; each block padded to 512
    col_start = np.zeros(SLOTS + 1, np.int64)
    blk_slots = []  # (s0, s1) slot range per block
    cur = 0
    s0 = 0
    for s in range(SLOTS):
        if cur % BLK + caps[s] > BLK:
            blk_slots.append((s0, s))
            s0 = s
            cur = (cur // BLK + 1) * BLK
        col_start[s] = cur
        cur += int(caps[s])
    blk_slots.append((s0, SLOTS))
    col_start[SLOTS] = cur
    NCOLS = int(cur)
    NBLK = len(blk_slots)
    N_cap = NBLK * BLK
    C_n = N_cap // 128

    # coef batches over block ranges (host + device must agree)
    bper = (NBLK + NBATCH - 1) // NBATCH
    batches = tuple((k * bper, min((k + 1) * bper, NBLK))
                    for k in range(NBATCH) if k * bper < NBLK)

    # positions per relation
    order_by_rel = np.argsort(rel, kind="stable")
    rel_sorted = rel[order_by_rel]
    starts = np.searchsorted(rel_sorted, np.arange(R2))
    ends = np.searchsorted(rel_sorted, np.arange(R2), side="right")

    pos_core = np.empty(N, np.int64)
    pos_col = np.empty(N, np.int64)

    # packed per-entity table (bf16): ent96 | yf mf df | yp mp dp | ya ma da
    tbl = np.zeros((NUM_ENT, 384), np.float32)
    tbl[:, 0:96] = np.asarray(inp["ent_embs"], np.float32)
    for k, nm in enumerate(["y_freq", "m_freq", "d_freq",
                            "y_phi", "m_phi", "d_phi",
                            "y_amp", "m_amp", "d_amp"]):
        tbl[:, 96 + 32 * k : 128 + 32 * k] = np.asarray(inp[nm], np.float32)
    tbl16 = tbl.astype(NPDT16)

    xrow_cores, tcat_cores = [], []
    for c in range(NC):
        gidx = np.zeros(N_cap, np.int64)
        valid = np.zeros(N_cap, bool)
        tc3 = np.zeros((N_cap, 3), np.float32)
        for s in range(SLOTS):
            r = core_rels[c][s]
            if r is None:
                continue
            pp = order_by_rel[starts[r] : ends[r]]
            pp = pp[np.argsort(nidx[pp], kind="stable")]
            j0 = int(col_start[s])
            gidx[j0 : j0 + len(pp)] = nidx[pp]
            valid[j0 : j0 + len(pp)] = True
            tc3[j0 : j0 + len(pp), 0] = years[pp]
            tc3[j0 : j0 + len(pp), 1] = months[pp]
            tc3[j0 : j0 + len(pp), 2] = days[pp]
            pos_core[pp] = c
            pos_col[pp] = j0 + np.arange(len(pp))
        xr = np.zeros((N_cap, 384), NPDT16)
        xr[valid] = tbl16[gidx[valid]]
        xrow_cores.append(_tile_rows(xr))
        tcat_cores.append(_tile_rows(tc3).astype(NPDT16))

    # ---- batch-local 0/1 expansion matrix [64, N_cap] ----
    Emat = np.zeros((64, N_cap), np.float32)
    for (b0, b1) in batches:
        S0 = blk_slots[b0][0]
        S1 = blk_slots[b1 - 1][1]
        assert S1 - S0 <= 64
        for s in range(S0, S1):
            Emat[s - S0, int(col_start[s]) : int(col_start[s]) + int(caps[s])] = 1.0
    p.Emat = np.ascontiguousarray(Emat.astype(NPDT16))

    # ---- per-core weights + BN aux ----
    wloc_cores, aux_cores = [], []
    for c in range(NC):
        wl = np.zeros((SLOTS, DIM, DIM), np.float32)
        invcnt = np.ones(SLOTS, np.float32)
        onem = np.ones(SLOTS, np.float32)
        gT_u = np.zeros((DIM, SLOTS), np.float32)
        ubb = np.zeros((DIM, SLOTS), np.float32)
        for s in range(SLOTS):
            r = core_rels[c][s]
            if r is None:
                continue
            wl[s] = W[r]
            cnt = cnts[r]
            invcnt[s] = 1.0 / max(cnt, 1)
            u = 1.0 if cnt > 1 else 0.0
            onem[s] = 1.0 - u
            gT_u[:, s] = gamma[r] * u
            ubb[:, s] = beta[r] * u + b[r] * (1.0 - u)
        aux = np.zeros((128, 4 * SLOTS), np.float32)
        aux[:, 0:SLOTS] = invcnt[None, :]
        aux[:, SLOTS : 2 * SLOTS] = onem[None, :]
        aux[:, 2 * SLOTS : 3 * SLOTS] = gT_u
        aux[:, 3 * SLOTS : 4 * SLOTS] = ubb
        wloc_cores.append(np.ascontiguousarray(
            wl.transpose(1, 0, 2).astype(NPDT16)))
        aux_cores.append(np.ascontiguousarray(aux))

    # ---- pooling edges: keep only dsts referenced by head/tail ----
    pcnt = np.bincount(pdst, minlength=Q).astype(np.float32)
    used = np.zeros(Q, bool)
    used[head] = True
    used[tail] = True
    keep = used[pdst]
    e_core = pos_core[psrc]
    ecols, edsts = [], []
    for c in range(NC):
        m = keep & (e_core == c)
        es, ed = psrc[m], pdst[m]
        o = np.argsort(pos_col[es], kind="stable")  # z_d locality + watermark
        ecols.append(pos_col[es[o]])
        edsts.append(ed[o])

    # dedup edge srcs: one gathered z row per distinct src, P rows summed
    uniq_cores = [np.unique(x) for x in ecols]
    T_E = max(1, max((len(u) + 127) // 128 for u in uniq_cores))
    NE = T_E * 128

    egidx_cores, p_cores = [], []
    tile_ready = np.zeros(T_E, np.int64)  # z_d chunks needed per edge tile
    for c in range(NC):
        ec, ed = ecols[c], edsts[c]
        uniq = uniq_cores[c]
        rows = np.searchsorted(uniq, ec)
        inv = 1.0 / np.maximum(pcnt[ed], 1.0)
        contrib = ((ed[:, None] == head[None, :]).astype(np.float32)
                   - (ed[:, None] == tail[None, :]).astype(np.float32))
        contrib *= inv[:, None]
        P = np.zeros((NE, B), np.float32)
        np.add.at(P, rows, contrib)
        eg = np.zeros(NE, np.int64)
        eg[: len(uniq)] = uniq
        egidx_cores.append(_wrap16(eg, NE))
        p_cores.append(np.ascontiguousarray(
            P.reshape(T_E, 128, B).transpose(1, 0, 2).astype(NPDT16)))
        hi = eg.reshape(T_E, 128).max(axis=1)  # cols sorted -> per-tile max
        tile_ready = np.maximum(tile_ready, hi // 128 + 1)
    p.tile_ready = tuple(int(x) for x in tile_ready)

    # ---- scoring: rel_embs[rels]^T / NC, folded pre-reduce ----
    # Butterfly exchange uses XOR-local triple blocks: on core c, local
    # 128-col block j holds global block c ^ j.  Permute P columns and
    # relgT per core accordingly; core 0's local order is then global.
    relgT = np.asarray(inp["rel_embs"], np.float32)[rels].T / NC  # [128, B]
    p.relgT = np.ascontiguousarray(relgT.astype(NPDT16))
    p.ident = np.ascontiguousarray(np.eye(128, dtype=NPDT16))

    p.SLOTS, p.caps, p.col_start = SLOTS, caps, col_start
    p.NCOLS, p.N_cap, p.C_n, p.T_E = NCOLS, N_cap, C_n, T_E
    p.NBLK, p.blk_slots, p.batches = NBLK, tuple(blk_slots), batches
    p.xrow, p.tcat = xrow_cores, tcat_cores
    p.wloc, p.aux = wloc_cores, aux_cores
    p.egidx, p.P = egidx_cores, p_cores
    return p


def _build(SLOTS, caps, col_start, N_cap, C_n, T_E, tile_ready, blk_slots,
           batches):
    NBLK = len(blk_slots)
    nc = bacc.Bacc(None, target_bir_lowering=False, debug=False,
                   num_devices=NC, num_swdge_queues=2)
    xrow = nc.dram_tensor("xrow", [128, C_n, 384], DT16, kind="ExternalInput")
    tcat = nc.dram_tensor("tcat", [128, C_n, 3], DT16, kind="ExternalInput")
    wloc = nc.dram_tensor("wloc", [128, SLOTS, DIM], DT16,
                          kind="ExternalInput")
    aux = nc.dram_tensor("aux", [128, 4 * SLOTS], F32, kind="ExternalInput")
    emat = nc.dram_tensor("emat", [64, N_cap], DT16, kind="ExternalInput")
    egidx = nc.dram_tensor("egidx", [128, T_E * 128 // 16], I16,
                           kind="ExternalInput")
    p_d = nc.dram_tensor("p_d", [128, T_E, B], DT16, kind="ExternalInput")
    relgT = nc.dram_tensor("relgT", [128, B], DT16, kind="ExternalInput")
    identd = nc.dram_tensor("identd", [128, 128], DT16, kind="ExternalInput")
    outd = nc.dram_tensor("out", [1, B], F32, kind="ExternalOutput")

    z_d = nc.dram_tensor("z_d", [N_cap, DIM], DT16)
    ard_in = nc.dram_tensor("ard_in", [128, B], DT16)
    ard_out = nc.dram_tensor("ard_out", [128, B], DT16, addr_space="Shared")
    war_in = nc.dram_tensor("war_in", [128, 16], DT16)
    war_out = nc.dram_tensor("war_out", [128, 16], DT16, addr_space="Shared")

    GG = 8  # chunks per x-load / time-embedding group
    n_gg = (C_n + GG - 1) // GG


    with tile.TileContext(nc) as tc:
        with (
            tc.tile_pool(name="pers", bufs=1) as sm,
            tc.tile_pool(name="ps", bufs=2, space="PSUM") as ps,
            tc.tile_pool(name="ptp", bufs=2, space="PSUM") as ptp,
            tc.tile_pool(name="pep", bufs=2, space="PSUM") as pep,
            tc.tile_pool(name="dtp", bufs=1, space="PSUM") as dtp,
            tc.tile_pool(name="ph2", bufs=1) as bg2,
        ):
            nc.gpsimd.load_library(library_config.mlp)

            def load(pool, name, dram, shape, dtype=F32):
                t = pool.tile(shape, dtype, tag=name, name=name)
                nc.sync.dma_start(out=t[:], in_=dram[:])
                return t

            # warm-up collective: pays the CC firmware spin-up cost and
            # re-synchronizes the cores early, off the critical path
            wu = sm.tile([128, 16], DT16, tag="wu")
            nc.vector.memset(wu[:], 0.0)
            nc.sync.dma_start(out=war_in[:], in_=wu[:])
            nc.gpsimd.collective_compute(
                "AllReduce", OP.add,
                replica_groups=[list(range(NC))],
                ins=[war_in.ap().opt()], outs=[war_out.ap().opt()])
            # x rows arrive per 8-chunk group (first-needed-first DMA order)
            # load order = first-needed-first: the tiny tcat/ident loads
            # must not queue behind 3.5MB of xrow data (that delayed the
            # first time-embedding group by ~8us)
            xr_t = sm.tile([128, C_n, 384], DT16, tag="xr")

            def xr_load(g):
                c0, c1 = g * GG, min((g + 1) * GG, C_n)
                nc.sync.dma_start(out=xr_t[:, c0:c1, :],
                                  in_=xrow[:, c0:c1, :])

            xr_load(0)
            tcat_t = load(sm, "tcat", tcat, [128, C_n, 3], DT16)
            ident16 = load(sm, "identd", identd, [128, 128], DT16)
            xr_load(1)
            w_sb = load(sm, "w_sb", wloc, [128, SLOTS, DIM], DT16)
            for g in range(2, n_gg):
                xr_load(g)
            aux_t = load(sm, "aux", aux, [128, 4 * SLOTS])
            emat_t = load(sm, "emat", emat, [64, N_cap], DT16)
            egidx_t = load(sm, "egidx", egidx, [128, T_E * 128 // 16], I16)
            p_sb = load(sm, "p_sb", p_d, [128, T_E, B], DT16)
            relgT_t = load(sm, "relgT", relgT, [128, B], DT16)

            xT = sm.tile([128, N_cap], DT16)
            z_sb = sm.tile([128, N_cap], DT16)
            znT = sm.tile([128, N_cap], DT16)
            stats6 = sm.tile([128, SLOTS, 6], F32)
            a_t = sm.tile([128, SLOTS], F32, tag="a_t")
            c_t = sm.tile([128, SLOTS], F32, tag="c_t")
            a16 = sm.tile([128, 128], DT16, tag="a16")
            c16 = sm.tile([128, 128], DT16, tag="c16")
            nc.vector.memset(a16[:], 0.0)
            nc.vector.memset(c16[:], 0.0)
            sc = [sm.tile([128, SLOTS], F32, tag=f"sc{i}", name=f"sc{i}")
                  for i in range(6)]
            V = nc.vector

            dt0 = dtp.tile([128, 512], F32, tag="dt0", space="PSUM",
                           name="dt0")
            dt1 = dtp.tile([128, 512], F32, tag="dt1", space="PSUM",
                           name="dt1")

            # ---- helpers ------------------------------------------------
            cp_eng = [0]

            def next_copy(out, in_):
                # rotate PSUM->SBUF copies over ACT / DVE (gpsimd can't
                # read PSUM)
                k = cp_eng[0] % 2
                cp_eng[0] += 1
                if k == 0:
                    nc.scalar.copy(out=out, in_=in_)
                else:
                    nc.vector.tensor_copy(out=out, in_=in_)

            def temb_group(g):
                # time embedding for chunks [c0,c1): x[:, :, 96:128] final
                c0, c1 = g * GG, min((g + 1) * GG, C_n)
                w = c1 - c0
                gb = xr_t[:, c0:c1, :]
                xs = bg2.tile([128, GG, 96], DT16, tag="xs", name="xs",
                              bufs=2)
                f4 = gb[:, :, 96:192].rearrange("p c (k e) -> p c k e", k=3)
                x4 = xs[:, :w, :].rearrange("p c (k e) -> p c k e", k=3)
                t4 = tcat_t[:, c0:c1, :].unsqueeze(3).to_broadcast(
                    [128, w, 3, T_DIM])
                V.tensor_tensor(out=x4, in0=f4, in1=t4, op=OP.mult)
                V.tensor_tensor(out=xs[:, :w, :], in0=xs[:, :w, :],
                                in1=gb[:, :, 192:288], op=OP.add)
                nc.scalar.activation(out=xs[:, :w, :], in_=xs[:, :w, :],
                                     func=AF.Sin)
                V.tensor_tensor(out=xs[:, :w, :], in0=xs[:, :w, :],
                                in1=gb[:, :, 288:384], op=OP.mult)
                V.tensor_tensor(out=gb[:, :, 96:128], in0=xs[:, :w, 0:32],
                                in1=xs[:, :w, 32:64], op=OP.add)
                V.tensor_tensor(out=gb[:, :, 96:128], in0=gb[:, :, 96:128],
                                in1=xs[:, :w, 64:96], op=OP.add)
                for c in range(c0, c1):
                    pt = ptp.tile([128, 128], DT16, tag="pt", space="PSUM",
                                  name="pt")
                    nc.tensor.transpose(out=pt[:], in_=gb[:, c - c0, 0:128],
                                        identity=ident16[:])
                    next_copy(xT[:, c * 128:(c + 1) * 128], pt[:])

            def pass1_block(b):
                # per-slot GEMM into one shared PSUM bank + stats + z save
                s0, s1 = blk_slots[b]
                base = b * BLK
                zp = ps.tile([128, BLK], F32, tag="zp", space="PSUM",
                             name="zp")
                for s in range(s0, s1):
                    a = int(col_start[s])
                    bb = a + int(caps[s])
                    if s == s1 - 1:
                        bb = base + BLK  # cover block pad (xT cols are zero)
                    nc.tensor.matmul(zp[:, a - base:bb - base],
                                     lhsT=w_sb[:, s, :], rhs=xT[:, a:bb],
                                     start=True, stop=True)
                for s in range(s0, s1):
                    a = int(col_start[s])
                    bb = a + int(caps[s])
                    nc.vector.bn_stats(stats6[:, s, :],
                                       zp[:, a - base:bb - base])
                V.tensor_copy(out=z_sb[:, base:base + BLK], in_=zp[:])

            def coefs(s0, s1):
                # BN coefs a,c for slots [s0,s1) + batch-local bf16 pack
                sl = slice(s0, s1)
                ce, me, ve = (stats6[:, sl, k] for k in (0, 1, 2))
                co, mo, vo = (stats6[:, sl, k] for k in (3, 4, 5))
                invcnt = aux_t[:, s0:s1]
                onem = aux_t[:, SLOTS + s0:SLOTS + s1]
                gT_u = aux_t[:, 2 * SLOTS + s0:2 * SLOTS + s1]
                ubb = aux_t[:, 3 * SLOTS + s0:3 * SLOTS + s1]
                te, to_, s1_, s2, mean, var = (t[:, sl] for t in sc)
                V.tensor_tensor(out=te, in0=ce, in1=me, op=OP.mult)
                V.tensor_tensor(out=to_, in0=co, in1=mo, op=OP.mult)
                V.tensor_tensor(out=s1_, in0=te, in1=to_, op=OP.add)
                V.tensor_tensor(out=s2, in0=ve, in1=vo, op=OP.add)
                V.tensor_tensor(out=te, in0=te, in1=me, op=OP.mult)
                V.tensor_tensor(out=s2, in0=s2, in1=te, op=OP.add)
                V.tensor_tensor(out=to_, in0=to_, in1=mo, op=OP.mult)
                V.tensor_tensor(out=s2, in0=s2, in1=to_, op=OP.add)
                V.tensor_tensor(out=mean, in0=s1_, in1=invcnt, op=OP.mult)
                V.tensor_tensor(out=s2, in0=s2, in1=invcnt, op=OP.mult)
                V.tensor_tensor(out=var, in0=mean, in1=mean, op=OP.mult)
                V.tensor_tensor(out=var, in0=s2, in1=var, op=OP.subtract)
                V.tensor_scalar(out=var, in0=var, scalar1=0.0,
                                scalar2=BN_EPS, op0=OP.max, op1=OP.add)
                nc.scalar.activation(out=var, in_=var, func=AF.Sqrt)
                V.reciprocal(out=var, in_=var)  # := 1/sqrt(var+eps)
                V.tensor_tensor(out=te, in0=gT_u, in1=var, op=OP.mult)
                V.tensor_tensor(out=a_t[:, sl], in0=te, in1=onem, op=OP.add)
                V.tensor_tensor(out=to_, in0=mean, in1=te, op=OP.mult)
                V.tensor_tensor(out=c_t[:, sl], in0=ubb, in1=to_,
                                op=OP.subtract)
                V.tensor_copy(out=a16[:, 0:s1 - s0], in_=a_t[:, sl])
                V.tensor_copy(out=c16[:, 0:s1 - s0], in_=c_t[:, sl])

            # pooling side: zn transposes -> z_d + edge gathers + D matmuls
            # (gathers fire as soon as their z_d watermark is met; gpsimd
            # is otherwise idle)
            state = {"wc": 0, "et": 0}
            GP = 4

            def edges_upto(final=False):
                while state["et"] < T_E:
                    t0 = state["et"]
                    t1 = t0
                    while (t1 < T_E and t1 - t0 < GP
                           and tile_ready[t1] <= state["wc"]):
                        t1 += 1
                    if t1 == t0 or (t1 - t0 < GP and not final):
                        break
                    wt = t1 - t0
                    zg = bg2.tile([128, GP, DIM], DT16, tag="zg", name="zg",
                                  bufs=2)
                    nc.gpsimd.dma_gather(
                        out_ap=zg[:, :wt, :],
                        in_ap=z_d[0:state["wc"] * 128, :],
                        idxs_ap=egidx_t[:, t0 * 8:t1 * 8],
                        num_idxs=wt * 128, num_idxs_reg=wt * 128,
                        elem_size=DIM, single_packet=True)
                    for t in range(t0, t1):
                        nc.tensor.matmul(dt0[:], lhsT=zg[:, t - t0, :],
                                         rhs=p_sb[:, t, 0:512],
                                         start=(t == 0), stop=(t == T_E - 1))
                        nc.tensor.matmul(dt1[:], lhsT=zg[:, t - t0, :],
                                         rhs=p_sb[:, t, 512:B],
                                         start=(t == 0), stop=(t == T_E - 1))
                    state["et"] = t1

            def pass2_block(b, aT, cT, ns):
                # zn = relu(a_exp*z + c_exp) for block b; transpose chunks
                # -> z rows -> z_d flush -> edge gathers + D matmuls
                base = b * BLK
                a_ex = pep.tile([128, BLK], F32, tag="ex", space="PSUM",
                                name="a_ex")
                c_ex = pep.tile([128, BLK], F32, tag="ex", space="PSUM",
                                name="c_ex")
                nc.tensor.matmul(a_ex[:], lhsT=aT[0:ns, :],
                                 rhs=emat_t[0:ns, base:base + BLK],
                                 start=True, stop=True)
                nc.tensor.matmul(c_ex[:], lhsT=cT[0:ns, :],
                                 rhs=emat_t[0:ns, base:base + BLK],
                                 start=True, stop=True)
                zb = znT[:, base:base + BLK]
                V.tensor_tensor(out=zb, in0=z_sb[:, base:base + BLK],
                                in1=a_ex[:], op=OP.mult)
                V.tensor_tensor(out=zb, in0=zb, in1=c_ex[:], op=OP.add)
                V.tensor_scalar_max(out=zb, in0=zb, scalar1=0.0)
                zr = bg2.tile([128, BCH, DIM], DT16, tag="zr", name="zr",
                              bufs=3)
                for c in range(BCH):
                    ch = base // 128 + c
                    pt = ptp.tile([128, 128], DT16, tag="pt", space="PSUM",
                                  name="pt")
                    nc.tensor.transpose(
                        out=pt[:], in_=znT[:, ch * 128:(ch + 1) * 128],
                        identity=ident16[:])
                    # one engine per block so the z_d flush has a single
                    # precise producer (avoids global-clock fallback waits)
                    if b % 2 == 0:
                        nc.scalar.copy(out=zr[:, c, :], in_=pt[:])
                    else:
                        nc.vector.tensor_copy(out=zr[:, c, :], in_=pt[:])
                nc.sync.dma_start(
                    out=z_d[base:base + BLK, :].rearrange(
                        "(c p) d -> p c d", p=128),
                    in_=zr[:])
                state["wc"] = base // 128 + BCH
                edges_upto(final=(b == NBLK - 1))

            # ---- main schedule -----------------------------------------
            done_g = 0
            done_p1 = 0

            def groups_upto(cn):
                nonlocal done_g
                while done_g < n_gg and done_g * GG < cn:
                    temb_group(done_g)
                    done_g += 1

            def pass1_upto(nb):
                nonlocal done_p1
                while done_p1 < nb:
                    groups_upto(min((done_p1 + 1) * BCH + GG, C_n))
                    pass1_block(done_p1)
                    done_p1 += 1

            # deferred-pass2 pipeline: batch k's coefs run on DVE while PE
            # executes pass2 of batch k-1 (kills the PE bubble at each
            # coef-batch boundary)
            pend = None
            for ki, (b0, b1) in enumerate(batches):
                pass1_upto(b1)
                S0 = blk_slots[b0][0]
                S1 = blk_slots[b1 - 1][1]
                ns = S1 - S0
                coefs(S0, S1)
                if ki + 1 < len(batches):
                    # keep PE busy on next batch's pass1 while DVE does coefs
                    pass1_upto(batches[ki + 1][1])
                if pend is not None:
                    for b in range(pend[0], pend[1]):
                        pass2_block(b, pend[2], pend[3], pend[4])
                ap_ = ptp.tile([128, 128], DT16, tag="pt", space="PSUM",
                               name="ap_")
                cp_ = ptp.tile([128, 128], DT16, tag="pt", space="PSUM",
                               name="cp_")
                nc.tensor.transpose(out=ap_[:], in_=a16[:],
                                    identity=ident16[:])
                nc.tensor.transpose(out=cp_[:], in_=c16[:],
                                    identity=ident16[:])
                aT = bg2.tile([128, 128], DT16, tag="aT", name="aT", bufs=2)
                cT = bg2.tile([128, 128], DT16, tag="cT", name="cT", bufs=2)
                nc.scalar.copy(out=aT[:], in_=ap_[:])
                nc.scalar.copy(out=cT[:], in_=cp_[:])
                pend = (b0, b1, aT, cT, ns)
            for b in range(pend[0], pend[1]):
                pass2_block(b, pend[2], pend[3], pend[4])

            # ---- AllReduce of D^T partials (relgT/NC folded) ----
            ones = sm.tile([128, 1], DT16)
            nc.vector.memset(ones[:], 1.0)
            dts = bg2.tile([128, B], DT16, tag="dts")
            nc.vector.tensor_tensor(out=dts[:, 0:512], in0=dt0[:],
                                    in1=relgT_t[:, 0:512], op=OP.add)
            nc.vector.tensor_tensor(out=dts[:, 512:B], in0=dt1[:],
                                    in1=relgT_t[:, 512:B], op=OP.add)
            nc.sync.dma_start(out=ard_in[:], in_=dts[:])
            nc.gpsimd.collective_compute(
                "AllReduce", OP.add,
                replica_groups=[list(range(NC))],
                ins=[ard_in.ap().opt()], outs=[ard_out.ap().opt()])

            # ---- scoring ----
            v_t = bg2.tile([128, B], DT16, tag="v_t")
            nc.sync.dma_start(out=v_t[:], in_=ard_out[:])
            sq = bg2.tile([128, B], DT16, tag="sq")
            nc.vector.tensor_tensor(out=sq[:], in0=v_t[:], in1=v_t[:],
                                    op=OP.mult)
            ss0 = ps.tile([1, 512], F32, tag="zp", space="PSUM", name="ss0")
            ss1 = ps.tile([1, 512], F32, tag="zp", space="PSUM", name="ss1")
            nc.tensor.matmul(ss0[:], lhsT=ones[:], rhs=sq[:, 0:512],
                             start=True, stop=True)
            nc.tensor.matmul(ss1[:], lhsT=ones[:], rhs=sq[:, 512:B],
                             start=True, stop=True)
            souts = bg2.tile([1, B], F32, tag="souts")
            nc.scalar.activation(out=souts[:, 0:512], in_=ss0[:],
                                 func=AF.Sqrt)
            nc.scalar.activation(out=souts[:, 512:B], in_=ss1[:],
                                 func=AF.Sqrt)
            nc.vector.tensor_scalar_mul(souts[:], souts[:], -1.0)
            nc.sync.dma_start(out=outd[:], in_=souts[:])

    nc.finalize()
    return nc


_CACHE = {}


def _in_maps(p):
    return [{
        "xrow": p.xrow[c],
        "tcat": p.tcat[c],
        "wloc": p.wloc[c],
        "aux": p.aux[c],
        "emat": p.Emat,
        "egidx": p.egidx[c],
        "p_d": p.P[c],
        "relgT": p.relgT,
        "identd": p.ident,
    } for c in range(NC)]


def kernel(**inputs) -> np.ndarray:
    p = _make_plan(inputs)
    key = (p.SLOTS, tuple(p.caps.tolist()), p.NCOLS, p.N_cap, p.C_n, p.T_E,
           p.tile_ready, p.blk_slots, p.batches)
    if key not in _CACHE:
        _CACHE[key] = _build(p.SLOTS, p.caps, p.col_start, p.N_cap,
                             p.C_n, p.T_E, p.tile_ready, p.blk_slots,
                             p.batches)
    nc = _CACHE[key]
    res = run_bass_kernel_spmd(nc, _in_maps(p), core_ids=list(range(NC)))
    return np.ascontiguousarray(res.results[0]["out"]).reshape(B).astype(
        np.float32)
